# revision 8
# baseline (speedup 1.0000x reference)
"""Distributed Bass kernel for attention-energy softmax on 8 TRN2 NeuronCores.

Computes: softmax(enc @ W.T @ h + (b.h)) == softmax(enc @ (W.T @ h)) over
S=32768.  The bias term b.h is a constant shift across all energies and
cancels in softmax, so b is unused.

Device does ONLY the memory-bound part: stream enc (fp16, 8 MiB/core) through
TensorE against a host-precomputed stationary v = W.T @ h, and DMA the raw
fp32 energies back.  Everything O(H^2) or O(S) scalar (v matvec, softmax
normalization) runs on host, as the sharding hint's distributed softmax
combine suggests.

Schedule facts this is built around (measured on this part):
- 2 HWDGE queues (sync/scalar), FIFO each; the 16 SDMA engines round-robin
  between queues at packet granularity -> each queue sustains ~205 GB/s,
  ~410 GB/s aggregate; a transfer's semaphore fires ~0.8-2 us after its
  last byte (HBM receipt round-trip).
- Sub-4KB partition lines pay a small-packet HBM penalty, so v is uploaded
  as ONE contiguous [1, 1024] fp16 line and transposed to [128, 8] on the
  PE (outer-product with [1,1] ones), not DMA'd as 128 16-byte lines.
- Slab sizes ramp 256,256 / 512 x6 / 256,256 (seq positions): small first
  so the first matmul group starts ~10us, 1 MiB in the middle for DMA
  efficiency, small last so the post-DMA matmul tail is ~1 us.

Per core (shard = 4096 seq positions, no cross-core sync):
  slab i covers seq [a, a+n): enc<i>[p, hc*n + jj] = enc[a+jj, hc*128+p],
  queues alternate sync/scalar (2048 seq each).  8 matmuls per slab
  (N=n) accumulate into PSUM (row 32*(a//1024), cols a%1024..) via
  tile_position; VectorE copies each finished [1,n] block to SBUF while
  later slabs stream; one 16 KiB gpsimd DMA returns [4, 1024] energies.
Host: global softmax over the gathered [32768] energies in float64.
"""

import sys

sys.path.insert(0, "/opt/trn_rl_repo")

import numpy as np

import concourse.bacc as bacc
import concourse.mybir as mybir
import concourse.tile as tile
from concourse.bass_utils import run_bass_kernel_spmd

N_CORES = 8
H = 1024
S = 32768
S_SHARD = S // N_CORES          # 4096
HC = H // 128                   # 8 h-chunks of 128 (contraction tiles)
FP32 = mybir.dt.float32
FP16 = mybir.dt.float16

# seq sizes per slab; none straddles a 512-seq PSUM bank window; alternating
# queues carry 2048 seq each
SLAB_SIZES = [128, 384, 512, 512, 512, 512, 512, 512, 384, 128]
assert sum(SLAB_SIZES) == S_SHARD
SLAB_STARTS = [sum(SLAB_SIZES[:i]) for i in range(len(SLAB_SIZES))]

_compiled_nc = None


def _build():
    nc = bacc.Bacc(
        "TRN2", target_bir_lowering=False, debug=False, num_devices=N_CORES
    )

    enc_ext = [
        nc.dram_tensor(f"enc{i}", [128, HC * n], FP16, kind="ExternalInput")
        for i, n in enumerate(SLAB_SIZES)
    ]
    vrow_ext = nc.dram_tensor("vrow", [1, H], FP16, kind="ExternalInput")
    # energies: row r holds seq [r*1024, (r+1)*1024) of this core's shard
    out_ext = nc.dram_tensor("out", [4, 1024], FP32, kind="ExternalOutput")

    with tile.TileContext(nc) as tc:
        with (
            tc.tile_pool(name="sb", bufs=1) as sb,
            tc.tile_pool(name="enc", bufs=len(SLAB_SIZES)) as encp,
            tc.tile_pool(name="ps", bufs=1, space="PSUM") as psp,
        ):
            vrow_sb = sb.tile([1, H], FP16, tag="vrow")
            one1 = sb.tile([1, 1], FP16, tag="one1")
            vcol_sb = sb.tile([128, HC], FP16, tag="vcol")
            scratch = sb.tile([128, 1024], FP32, tag="scr")

            nc.sync.dma_start(out=vrow_sb[:, :], in_=vrow_ext[:, :])
            nc.vector.memset(one1[:, :], 1.0)

            # transpose v to one [128,1] column per h-chunk: PE outer product
            vc_ps = psp.tile([128, HC], FP32, tag="vcps")
            for q in range(HC):
                nc.tensor.matmul(
                    vc_ps[:, q : q + 1],
                    lhsT=vrow_sb[0:1, q * 128 : (q + 1) * 128],
                    rhs=one1[0:1, 0:1],
                    start=True,
                    stop=True,
                )
            nc.vector.tensor_copy(vcol_sb[:, :], vc_ps[:, :])

            e_ps = psp.tile([128, 1024], FP32, tag="eps")
            for i, n in enumerate(SLAB_SIZES):
                a = SLAB_STARTS[i]
                slab = encp.tile([128, HC * n], FP16, tag="slab")
                nc.sync.dma_start(out=slab[:, :], in_=enc_ext[i][:, :])
                row = 32 * (a // 1024)
                col = a % 1024
                for hc in range(HC):
                    nc.tensor.matmul(
                        e_ps[row : row + 1, col : col + n],
                        lhsT=vcol_sb[:, hc : hc + 1],
                        rhs=slab[:, hc * n : (hc + 1) * n],
                        start=(hc == 0),
                        stop=(hc == HC - 1),
                        tile_position=(0, row),
                    )
                nc.vector.tensor_copy(
                    scratch[row : row + 1, col : col + n],
                    e_ps[row : row + 1, col : col + n],
                )
            nc.scalar.dma_start(
                out=out_ext[:, :], in_=scratch[0 : 3 * 32 + 1 : 32, :]
            )

    nc.compile()
    return nc


def get_nc():
    global _compiled_nc
    if _compiled_nc is None:
        _compiled_nc = _build()
    return _compiled_nc


def make_in_maps(hidden_state, encoder_output, W):
    h = np.asarray(hidden_state, dtype=np.float32).reshape(H)
    Wf = np.asarray(W, dtype=np.float32).reshape(H, H)
    vrow = (Wf.T @ h).astype(np.float16).reshape(1, H)

    enc16 = (
        np.asarray(encoder_output, dtype=np.float32)
        .reshape(S, H)
        .astype(np.float16)
    )
    in_maps = []
    for c in range(N_CORES):
        shard = enc16[c * S_SHARD : (c + 1) * S_SHARD]     # [4096, 1024]
        m = {"vrow": vrow}
        for i, n in enumerate(SLAB_SIZES):
            a = SLAB_STARTS[i]
            # enc<i>[p, hc*n + jj] = shard[a + jj, hc*128 + p]
            m[f"enc{i}"] = np.ascontiguousarray(
                shard[a : a + n].reshape(n, HC, 128).transpose(2, 1, 0)
            ).reshape(128, HC * n)
        in_maps.append(m)
    return in_maps


def unshard(results):
    # gather raw energies, exact softmax on host (f64)
    e = np.concatenate(
        [results[c]["out"].reshape(-1) for c in range(N_CORES)]
    ).astype(np.float64)                                   # [S]
    e -= e.max()
    w = np.exp(e)
    w /= w.sum()
    return w.astype(np.float32)[None, :]


def kernel(hidden_state, encoder_output, W, b=None, **_unused):
    nc = get_nc()
    in_maps = make_in_maps(hidden_state, encoder_output, W)
    res = run_bass_kernel_spmd(nc, in_maps, core_ids=list(range(N_CORES)))
    return unshard(res.results)


# revision 9
# speedup vs baseline: 1.1030x; 1.1030x over previous
"""Distributed Bass kernel for attention-energy softmax on 8 TRN2 NeuronCores.

Computes: softmax(enc @ W.T @ h + (b.h)) == softmax(enc @ (W.T @ h)) over
S=32768.  The bias term b.h is a constant shift across all energies and
cancels in softmax, so b is unused.

Device does ONLY the memory-bound part: stream enc (fp16, 8 MiB/core) through
TensorE against a host-precomputed stationary v = W.T @ h, and DMA the raw
fp32 energies back.  Everything O(H^2) or O(S) scalar (v matvec, softmax
normalization) runs on host, as the sharding hint's distributed softmax
combine suggests.

Schedule facts this is built around (measured on this part):
- 2 HWDGE queues (sync/scalar), FIFO each; the 16 SDMA engines round-robin
  between queues at packet granularity -> each queue sustains ~205 GB/s,
  ~410 GB/s aggregate; a transfer's semaphore fires ~0.8-2 us after its
  last byte (HBM receipt round-trip).
- Sub-4KB partition lines pay a small-packet HBM penalty, so v is uploaded
  as ONE contiguous [1, 1024] fp16 line and transposed to [128, 8] on the
  PE (outer-product with [1,1] ones), not DMA'd as 128 16-byte lines.
- Slab sizes ramp 256,256 / 512 x6 / 256,256 (seq positions): small first
  so the first matmul group starts ~10us, 1 MiB in the middle for DMA
  efficiency, small last so the post-DMA matmul tail is ~1 us.

Per core (shard = 4096 seq positions, no cross-core sync):
  slab i covers seq [a, a+n): enc<i>[p, hc*n + jj] = enc[a+jj, hc*128+p],
  queues alternate sync/scalar (2048 seq each).  8 matmuls per slab
  (N=n) accumulate into PSUM (row 32*(a//1024), cols a%1024..) via
  tile_position; VectorE copies each finished [1,n] block to SBUF while
  later slabs stream; one 16 KiB gpsimd DMA returns [4, 1024] energies.
Host: global softmax over the gathered [32768] energies in float64.
"""

import sys

sys.path.insert(0, "/opt/trn_rl_repo")

import numpy as np

import concourse.bacc as bacc
import concourse.mybir as mybir
import concourse.tile as tile
from concourse.bass_utils import run_bass_kernel_spmd

N_CORES = 8
H = 1024
S = 32768
S_SHARD = S // N_CORES          # 4096
HC = H // 128                   # 8 h-chunks of 128 (contraction tiles)
FP32 = mybir.dt.float32
FP16 = mybir.dt.float16

# seq sizes per slab; none straddles a 512-seq PSUM bank window; alternating
# queues carry 2048 seq each
SLAB_SIZES = [128, 384, 512, 512, 512, 512, 512, 512, 384, 128]
# slab i -> scalar when even, sync when odd (testing queue asymmetry)
assert sum(SLAB_SIZES) == S_SHARD
SLAB_STARTS = [sum(SLAB_SIZES[:i]) for i in range(len(SLAB_SIZES))]

_compiled_nc = None


def _build():
    nc = bacc.Bacc(
        "TRN2", target_bir_lowering=False, debug=False, num_devices=N_CORES
    )

    enc_ext = [
        nc.dram_tensor(f"enc{i}", [128, HC * n], FP16, kind="ExternalInput")
        for i, n in enumerate(SLAB_SIZES)
    ]
    vrow_ext = nc.dram_tensor("vrow", [1, H], FP16, kind="ExternalInput")
    # energies: row r holds seq [r*1024, (r+1)*1024) of this core's shard
    out_ext = nc.dram_tensor("out", [4, 1024], FP32, kind="ExternalOutput")

    with tile.TileContext(nc) as tc:
        with (
            tc.tile_pool(name="sb", bufs=1) as sb,
            tc.tile_pool(name="enc", bufs=len(SLAB_SIZES)) as encp,
            tc.tile_pool(name="ps", bufs=1, space="PSUM") as psp,
        ):
            vrow_sb = sb.tile([1, H], FP16, tag="vrow")
            one1 = sb.tile([1, 1], FP16, tag="one1")
            vcol_sb = sb.tile([128, HC], FP16, tag="vcol")
            scratch = sb.tile([128, 1024], FP32, tag="scr")

            nc.sync.dma_start(out=vrow_sb[:, :], in_=vrow_ext[:, :])
            nc.vector.memset(one1[:, :], 1.0)

            # transpose v to one [128,1] column per h-chunk: PE outer product
            vc_ps = psp.tile([128, HC], FP32, tag="vcps")
            for q in range(HC):
                nc.tensor.matmul(
                    vc_ps[:, q : q + 1],
                    lhsT=vrow_sb[0:1, q * 128 : (q + 1) * 128],
                    rhs=one1[0:1, 0:1],
                    start=True,
                    stop=True,
                )
            nc.vector.tensor_copy(vcol_sb[:, :], vc_ps[:, :])

            e_ps = psp.tile([128, 1024], FP32, tag="eps")
            for i, n in enumerate(SLAB_SIZES):
                a = SLAB_STARTS[i]
                slab = encp.tile([128, HC * n], FP16, tag="slab")
                eng = nc.scalar if i % 2 == 0 else nc.sync
                eng.dma_start(out=slab[:, :], in_=enc_ext[i][:, :])
                row = 32 * (a // 1024)
                col = a % 1024
                for hc in range(HC):
                    nc.tensor.matmul(
                        e_ps[row : row + 1, col : col + n],
                        lhsT=vcol_sb[:, hc : hc + 1],
                        rhs=slab[:, hc * n : (hc + 1) * n],
                        start=(hc == 0),
                        stop=(hc == HC - 1),
                        tile_position=(0, row),
                    )
                nc.vector.tensor_copy(
                    scratch[row : row + 1, col : col + n],
                    e_ps[row : row + 1, col : col + n],
                )
            nc.gpsimd.dma_start(
                out=out_ext[0:3, :], in_=scratch[0 : 2 * 32 + 1 : 32, :]
            )
            nc.scalar.dma_start(
                out=out_ext[3:4, :], in_=scratch[96:97, :]
            )

    nc.compile()
    return nc


def get_nc():
    global _compiled_nc
    if _compiled_nc is None:
        _compiled_nc = _build()
    return _compiled_nc


def make_in_maps(hidden_state, encoder_output, W):
    h = np.asarray(hidden_state, dtype=np.float32).reshape(H)
    Wf = np.asarray(W, dtype=np.float32).reshape(H, H)
    vrow = (Wf.T @ h).astype(np.float16).reshape(1, H)

    enc16 = (
        np.asarray(encoder_output, dtype=np.float32)
        .reshape(S, H)
        .astype(np.float16)
    )
    in_maps = []
    for c in range(N_CORES):
        shard = enc16[c * S_SHARD : (c + 1) * S_SHARD]     # [4096, 1024]
        m = {"vrow": vrow}
        for i, n in enumerate(SLAB_SIZES):
            a = SLAB_STARTS[i]
            # enc<i>[p, hc*n + jj] = shard[a + jj, hc*128 + p]
            m[f"enc{i}"] = np.ascontiguousarray(
                shard[a : a + n].reshape(n, HC, 128).transpose(2, 1, 0)
            ).reshape(128, HC * n)
        in_maps.append(m)
    return in_maps


def unshard(results):
    # gather raw energies, exact softmax on host (f64)
    e = np.concatenate(
        [results[c]["out"].reshape(-1) for c in range(N_CORES)]
    ).astype(np.float64)                                   # [S]
    e -= e.max()
    w = np.exp(e)
    w /= w.sum()
    return w.astype(np.float32)[None, :]


def kernel(hidden_state, encoder_output, W, b=None, **_unused):
    nc = get_nc()
    in_maps = make_in_maps(hidden_state, encoder_output, W)
    res = run_bass_kernel_spmd(nc, in_maps, core_ids=list(range(N_CORES)))
    return unshard(res.results)
